# revision 3
# baseline (speedup 1.0000x reference)
"""CLUB loss kernel for Trainium2, 8-core data-parallel SPMD.

Math: with flat_x (N,D) [from x (B,D,H,W) -> (B*H*W, D)], v = exp(-p_logvar),
  loss = mean_i[ -0.5*sum_d ((x-mu)^2 - (m2 - 2*mu*m1 + mu^2)) * v ]
       = (-0.5/N) * [ A - 2B - dot(m2, V) + 2*dot(m1, W) ]
where
  A  = sum_{i,d} x^2 v          B  = sum_{i,d} x mu v
  V_d = sum_i v                 W_d = sum_i mu v
  m1 = S1/N, m2 = S2/N,  S1_d = sum_i x,  S2_d = sum_i x^2
All terms are per-core-local partial sums; the tiny (~KB) cross-core
reduction and final dot products happen on host in float64. No collectives.

Per-core device work (shard = 2048 rows = 2 b-blocks):
  - load x native (d-major), transpose 128x128 blocks on PE into PSUM
  - ACT: v = exp(-lv), p = square(xT), S1 via Copy+accum on native x
  - DVE: w = v*mu, fused tensor_tensor_reduce for A and B row-partials
  - PE: ones-matmul column sums for V, W, S2 accumulated in PSUM
"""

import sys

import numpy as np

for _p in ("/opt/trn_rl_repo",):
    if _p not in sys.path:
        sys.path.append(_p)

B, D, H, W = 16, 512, 32, 32
HW = H * W
N = B * HW
NCORES = 8
BLKB = B // NCORES          # b-blocks per core (2)
ROWS = N // NCORES          # rows per core (2048)
NT = ROWS // 128            # 128-row tiles per core (16)
NDC = D // 128              # d chunks (4)

_prog_cache = {}


def build_program():
    import concourse.bacc as bacc
    import concourse.tile as tile
    from concourse import masks, mybir

    f32 = mybir.dt.float32
    AF = mybir.ActivationFunctionType
    OP = mybir.AluOpType

    nc = bacc.Bacc(
        "TRN2",
        target_bir_lowering=False,
        debug=False,
        enable_asserts=False,
        num_devices=NCORES,
    )

    x_d = nc.dram_tensor("x_s", (BLKB, D, HW), f32, kind="ExternalInput").ap()
    mu_d = nc.dram_tensor("mu_s", (ROWS, D), f32, kind="ExternalInput").ap()
    lv_d = nc.dram_tensor("lv_s", (ROWS, D), f32, kind="ExternalInput").ap()

    o_v = nc.dram_tensor("o_v", (1, D), f32, kind="ExternalOutput").ap()
    o_w = nc.dram_tensor("o_w", (1, D), f32, kind="ExternalOutput").ap()
    o_s2 = nc.dram_tensor("o_s2", (1, D), f32, kind="ExternalOutput").ap()
    o_s1 = nc.dram_tensor("o_s1", (128, BLKB * NDC), f32, kind="ExternalOutput").ap()
    o_ra = nc.dram_tensor("o_ra", (128, NT), f32, kind="ExternalOutput").ap()
    o_rb = nc.dram_tensor("o_rb", (128, NT), f32, kind="ExternalOutput").ap()

    with tile.TileContext(nc) as tc:
        with (
            tc.tile_pool(name="const", bufs=1) as constp,
            tc.tile_pool(name="xnat", bufs=1) as xp,
            tc.tile_pool(name="stream", bufs=3) as sp,
            tc.tile_pool(name="accum", bufs=1) as accp,
            tc.tile_pool(name="scr", bufs=2) as scrp,
            tc.tile_pool(name="psum", bufs=2, space="PSUM") as pp,
            tc.tile_pool(name="psacc", bufs=1, space="PSUM") as pacc,
        ):
            ident = constp.tile([128, 128], f32)
            masks.make_identity(nc, ident[:])
            ones = constp.tile([128, 1], f32)
            nc.vector.memset(ones[:], 1.0)

            v_acc = pacc.tile([1, D], f32, tag="v_acc")
            w_acc = pacc.tile([1, D], f32, tag="w_acc")
            s2_acc = pacc.tile([1, D], f32, tag="s2_acc")

            ra = accp.tile([128, NT], f32, tag="ra")
            rb = accp.tile([128, NT], f32, tag="rb")
            s1c = accp.tile([128, BLKB * NDC], f32, tag="s1c")

            # ---- load x native (d-major): 8 tiles of (128, HW) ----
            x_sb = []
            for b in range(BLKB):
                for dc in range(NDC):
                    t_ = xp.tile([128, HW], f32, tag=f"x_{b}_{dc}")
                    nc.sync.dma_start(t_[:], x_d[b, 128 * dc : 128 * (dc + 1), :])
                    x_sb.append(t_)

            # ---- S1: per-d sums of x over i, via ACT Copy + accum ----
            for k in range(BLKB * NDC):
                scr_nat = scrp.tile([128, HW], f32, tag="scr_nat")
                nc.scalar.activation(
                    scr_nat[:], x_sb[k][:], AF.Copy,
                    accum_out=s1c[:, k : k + 1],
                )

            # ---- main loop over 128-row i-tiles ----
            for t in range(NT):
                b, j = divmod(t, NT // BLKB)

                xT = pp.tile([128, D], f32, tag="xT")
                for dc in range(NDC):
                    # 4 chunks share one PSUM bank: one accumulation group
                    nc.tensor.matmul(
                        xT[:, 128 * dc : 128 * (dc + 1)],
                        x_sb[b * NDC + dc][:, 128 * j : 128 * (j + 1)],
                        ident[:],
                        is_transpose=True,
                        start=(dc == 0),
                        stop=(dc == NDC - 1),
                    )

                mu_t = sp.tile([128, D], f32, tag="mu")
                nc.sync.dma_start(mu_t[:], mu_d[128 * t : 128 * (t + 1), :])
                lv_t = sp.tile([128, D], f32, tag="lv")
                nc.sync.dma_start(lv_t[:], lv_d[128 * t : 128 * (t + 1), :])

                v_t = sp.tile([128, D], f32, tag="v")
                nc.scalar.activation(v_t[:], lv_t[:], AF.Exp, scale=-1.0)
                p_t = sp.tile([128, D], f32, tag="p")
                nc.scalar.activation(p_t[:], xT[:], AF.Square)

                w_t = sp.tile([128, D], f32, tag="w")
                nc.vector.tensor_tensor(w_t[:], v_t[:], mu_t[:], OP.mult)

                scr_a = scrp.tile([128, D], f32, tag="scr_a")
                nc.vector.scalar_tensor_tensor(
                    out=scr_a[:], in0=p_t[:], scalar=1.0, in1=v_t[:],
                    op0=OP.mult, op1=OP.mult,
                    accum_out=ra[:, t : t + 1],
                )
                scr_b = scrp.tile([128, D], f32, tag="scr_b")
                nc.vector.scalar_tensor_tensor(
                    out=scr_b[:], in0=w_t[:], scalar=1.0, in1=xT[:],
                    op0=OP.mult, op1=OP.mult,
                    accum_out=rb[:, t : t + 1],
                )

                nc.tensor.matmul(
                    v_acc[:], ones[:], v_t[:], start=(t == 0), stop=(t == NT - 1)
                )
                nc.tensor.matmul(
                    w_acc[:], ones[:], w_t[:], start=(t == 0), stop=(t == NT - 1)
                )
                nc.tensor.matmul(
                    s2_acc[:], ones[:], p_t[:], start=(t == 0), stop=(t == NT - 1)
                )

            # ---- evacuate PSUM rows -> SBUF -> DRAM; SBUF accs -> DRAM ----
            v_row = accp.tile([1, D], f32, tag="v_row")
            nc.scalar.copy(v_row[:], v_acc[:])
            w_row = accp.tile([1, D], f32, tag="w_row")
            nc.scalar.copy(w_row[:], w_acc[:])
            s2_row = accp.tile([1, D], f32, tag="s2_row")
            nc.scalar.copy(s2_row[:], s2_acc[:])

            nc.sync.dma_start(o_v, v_row[:])
            nc.sync.dma_start(o_w, w_row[:])
            nc.sync.dma_start(o_s2, s2_row[:])
            nc.sync.dma_start(o_s1, s1c[:])
            nc.sync.dma_start(o_ra, ra[:])
            nc.sync.dma_start(o_rb, rb[:])

    nc.compile()
    return nc


def get_program():
    if "nc" not in _prog_cache:
        _prog_cache["nc"] = build_program()
    return _prog_cache["nc"]


def make_in_maps(x, p_mu, p_logvar):
    x = np.ascontiguousarray(np.asarray(x, dtype=np.float32)).reshape(B, D, HW)
    p_mu = np.ascontiguousarray(np.asarray(p_mu, dtype=np.float32))
    p_logvar = np.ascontiguousarray(np.asarray(p_logvar, dtype=np.float32))
    in_maps = []
    for c in range(NCORES):
        in_maps.append(
            {
                "x_s": np.ascontiguousarray(x[BLKB * c : BLKB * (c + 1)]),
                "mu_s": np.ascontiguousarray(p_mu[ROWS * c : ROWS * (c + 1)]),
                "lv_s": np.ascontiguousarray(p_logvar[ROWS * c : ROWS * (c + 1)]),
            }
        )
    return in_maps


def finish_host(results):
    """Combine per-core partials (float64) into the scalar loss."""
    Vv = np.zeros(D)
    Ww = np.zeros(D)
    S2 = np.zeros(D)
    S1 = np.zeros(D)
    A = 0.0
    Bb = 0.0
    for r in results:
        Vv += r["o_v"].astype(np.float64)[0]
        Ww += r["o_w"].astype(np.float64)[0]
        S2 += r["o_s2"].astype(np.float64)[0]
        s1c = r["o_s1"].astype(np.float64)
        for b in range(BLKB):
            for dc in range(NDC):
                S1[128 * dc : 128 * (dc + 1)] += s1c[:, b * NDC + dc]
        A += float(r["o_ra"].astype(np.float64).sum())
        Bb += float(r["o_rb"].astype(np.float64).sum())
    m1 = S1 / N
    m2 = S2 / N
    S = A - 2.0 * Bb - float(np.dot(m2, Vv)) + 2.0 * float(np.dot(m1, Ww))
    return np.float32(-0.5 / N * S)


def run_on_device(x, p_mu, p_logvar, trace=False, **kw):
    from concourse import bass_utils

    nc = get_program()
    in_maps = make_in_maps(x, p_mu, p_logvar)
    return bass_utils.run_bass_kernel_spmd(
        nc, in_maps, list(range(NCORES)), trace=trace, **kw
    )


def kernel(x, p_mu, p_logvar):
    res = run_on_device(x, p_mu, p_logvar)
    return finish_host(res.results)


# revision 10
# speedup vs baseline: 1.0370x; 1.0370x over previous
"""CLUB loss kernel for Trainium2, 8-core data-parallel SPMD.

Math: with flat_x (N,D) [from x (B,D,H,W) -> (B*H*W, D)], v = exp(-p_logvar),
  loss = mean_i[ -0.5*sum_d ((x-mu)^2 - (m2 - 2*mu*m1 + mu^2)) * v ]
       = (-0.5/N) * [ A - 2B - dot(m2, V) + 2*dot(m1, W) ]
where
  A  = sum_{i,d} x^2 v          B  = sum_{i,d} x mu v
  V_d = sum_i v                 W_d = sum_i mu v
  m1 = S1/N, m2 = S2/N,  S1_d = sum_i x,  S2_d = sum_i x^2
All terms are per-core-local partial sums; the tiny (~KB) cross-core
reduction and final dot products happen on host in float64. No collectives.

Per-core device work (shard = 2048 rows = 2 b-blocks):
  - load x native (d-major), transpose 128x128 blocks on PE into PSUM
  - ACT: v = exp(-lv), p = square(xT), S1 via Copy+accum on native x
  - DVE: w = v*mu, fused tensor_tensor_reduce for A and B row-partials
  - PE: ones-matmul column sums for V, W, S2 accumulated in PSUM
"""

import sys

import numpy as np

for _p in ("/opt/trn_rl_repo",):
    if _p not in sys.path:
        sys.path.append(_p)

B, D, H, W = 16, 512, 32, 32
HW = H * W
N = B * HW
NCORES = 8
BLKB = B // NCORES          # b-blocks per core (2)
ROWS = N // NCORES          # rows per core (2048)
NT = ROWS // 128            # 128-row tiles per core (16)
NDC = D // 128              # d chunks (4)

_prog_cache = {}


def build_program():
    import concourse.bacc as bacc
    import concourse.tile as tile
    from concourse import masks, mybir

    f32 = mybir.dt.float32
    AF = mybir.ActivationFunctionType
    OP = mybir.AluOpType

    nc = bacc.Bacc(
        "TRN2",
        target_bir_lowering=False,
        debug=False,
        enable_asserts=False,
        num_devices=NCORES,
    )

    x_d = nc.dram_tensor("x_s", (BLKB, D, HW), f32, kind="ExternalInput").ap()
    mu_d = nc.dram_tensor("mu_s", (ROWS, D), f32, kind="ExternalInput").ap()
    lv_d = nc.dram_tensor("lv_s", (ROWS, D), f32, kind="ExternalInput").ap()

    o_v = nc.dram_tensor("o_v", (1, D), f32, kind="ExternalOutput").ap()
    o_w = nc.dram_tensor("o_w", (1, D), f32, kind="ExternalOutput").ap()
    o_s2 = nc.dram_tensor("o_s2", (1, D), f32, kind="ExternalOutput").ap()
    o_s1 = nc.dram_tensor("o_s1", (128, BLKB * NDC), f32, kind="ExternalOutput").ap()
    o_ra = nc.dram_tensor("o_ra", (128, NT), f32, kind="ExternalOutput").ap()
    o_rb = nc.dram_tensor("o_rb", (128, NT), f32, kind="ExternalOutput").ap()

    with tile.TileContext(nc) as tc:
        with (
            tc.tile_pool(name="const", bufs=1) as constp,
            tc.tile_pool(name="xnat", bufs=1) as xp,
            tc.tile_pool(name="stream", bufs=3) as sp,
            tc.tile_pool(name="accum", bufs=1) as accp,
            tc.tile_pool(name="scr", bufs=2) as scrp,
            tc.tile_pool(name="psum", bufs=2, space="PSUM") as pp,
            tc.tile_pool(name="psacc", bufs=1, space="PSUM") as pacc,
        ):
            ident = constp.tile([128, 128], f32)
            masks.make_identity(nc, ident[:])
            ones = constp.tile([128, 1], f32)
            nc.vector.memset(ones[:], 1.0)

            # V/W/S2 accumulator rows live at partitions 0/32/64 of ONE psum
            # bank so the three ones-matmuls run concurrently in separate PE
            # column groups (tile_position) — 3x cheaper than serial fp32.
            acc3 = pacc.tile([65, D], f32, tag="acc3")

            ra = accp.tile([128, NT], f32, tag="ra")
            rb = accp.tile([128, NT], f32, tag="rb")
            s1c = accp.tile([128, BLKB * NDC], f32, tag="s1c")

            # ---- load x native (d-major): 8 tiles of (128, HW) ----
            x_sb = []
            for b in range(BLKB):
                for dc in range(NDC):
                    t_ = xp.tile([128, HW], f32, tag=f"x_{b}_{dc}")
                    nc.sync.dma_start(t_[:], x_d[b, 128 * dc : 128 * (dc + 1), :])
                    x_sb.append(t_)

            # ---- S1: per-d sums of x over i, via ACT Copy + accum ----
            for k in range(BLKB * NDC):
                scr_nat = scrp.tile([128, HW], f32, tag="scr_nat")
                nc.scalar.activation(
                    scr_nat[:], x_sb[k][:], AF.Copy,
                    accum_out=s1c[:, k : k + 1],
                )

            # ---- main loop over 128-row i-tiles ----
            for t in range(NT):
                b, j = divmod(t, NT // BLKB)

                xT = pp.tile([128, D], f32, tag="xT")
                for dc in range(NDC):
                    # 4 chunks share one PSUM bank: one accumulation group
                    nc.tensor.matmul(
                        xT[:, 128 * dc : 128 * (dc + 1)],
                        x_sb[b * NDC + dc][:, 128 * j : 128 * (j + 1)],
                        ident[:],
                        is_transpose=True,
                        start=(dc == 0),
                        stop=(dc == NDC - 1),
                    )

                mu_t = sp.tile([128, D], f32, tag="mu")
                nc.sync.dma_start(mu_t[:], mu_d[128 * t : 128 * (t + 1), :])
                lv_t = sp.tile([128, D], f32, tag="lv")
                nc.sync.dma_start(lv_t[:], lv_d[128 * t : 128 * (t + 1), :])

                v_t = sp.tile([128, D], f32, tag="v")
                nc.scalar.activation(v_t[:], lv_t[:], AF.Exp, scale=-1.0)
                p_t = sp.tile([128, D], f32, tag="p")
                nc.scalar.activation(p_t[:], xT[:], AF.Square)

                # w on GPSIMD (otherwise idle) to keep DVE at two passes/tile
                w_t = sp.tile([128, D], f32, tag="w")
                nc.gpsimd.tensor_tensor(w_t[:], v_t[:], mu_t[:], OP.mult)

                scr_a = scrp.tile([128, D], f32, tag="scr_a")
                nc.vector.scalar_tensor_tensor(
                    out=scr_a[:], in0=p_t[:], scalar=1.0, in1=v_t[:],
                    op0=OP.mult, op1=OP.mult,
                    accum_out=ra[:, t : t + 1],
                )
                scr_b = scrp.tile([128, D], f32, tag="scr_b")
                nc.vector.scalar_tensor_tensor(
                    out=scr_b[:], in0=w_t[:], scalar=1.0, in1=xT[:],
                    op0=OP.mult, op1=OP.mult,
                    accum_out=rb[:, t : t + 1],
                )

                # psum group state is per output partition: each row keeps its
                # own start/stop; col-groups 0/32/64 execute concurrently
                st, sp_ = (t == 0), (t == NT - 1)
                nc.tensor.matmul(
                    acc3[0:1, :], ones[:], v_t[:],
                    start=st, stop=sp_, tile_position=(0, 0),
                )
                nc.tensor.matmul(
                    acc3[32:33, :], ones[:], w_t[:],
                    start=st, stop=sp_, tile_position=(0, 32),
                )
                nc.tensor.matmul(
                    acc3[64:65, :], ones[:], p_t[:],
                    start=st, stop=sp_, tile_position=(0, 64),
                )

            # ---- evacuate PSUM rows -> SBUF -> DRAM; SBUF accs -> DRAM ----
            # compute engines are lane-locked, so each row evacuates to the
            # same partition index in SBUF; DMA moves them to row 0 in DRAM.
            rows_sb = accp.tile([65, D], f32, tag="rows_sb")
            nc.scalar.copy(rows_sb[0:1, :], acc3[0:1, :])
            nc.scalar.copy(rows_sb[32:33, :], acc3[32:33, :])
            nc.scalar.copy(rows_sb[64:65, :], acc3[64:65, :])

            nc.sync.dma_start(o_v, rows_sb[0:1, :])
            nc.sync.dma_start(o_w, rows_sb[32:33, :])
            nc.sync.dma_start(o_s2, rows_sb[64:65, :])
            nc.sync.dma_start(o_s1, s1c[:])
            nc.sync.dma_start(o_ra, ra[:])
            nc.sync.dma_start(o_rb, rb[:])

    nc.compile()
    return nc


def get_program():
    if "nc" not in _prog_cache:
        _prog_cache["nc"] = build_program()
    return _prog_cache["nc"]


def make_in_maps(x, p_mu, p_logvar):
    x = np.ascontiguousarray(np.asarray(x, dtype=np.float32)).reshape(B, D, HW)
    p_mu = np.ascontiguousarray(np.asarray(p_mu, dtype=np.float32))
    p_logvar = np.ascontiguousarray(np.asarray(p_logvar, dtype=np.float32))
    in_maps = []
    for c in range(NCORES):
        in_maps.append(
            {
                "x_s": np.ascontiguousarray(x[BLKB * c : BLKB * (c + 1)]),
                "mu_s": np.ascontiguousarray(p_mu[ROWS * c : ROWS * (c + 1)]),
                "lv_s": np.ascontiguousarray(p_logvar[ROWS * c : ROWS * (c + 1)]),
            }
        )
    return in_maps


def finish_host(results):
    """Combine per-core partials (float64) into the scalar loss."""
    Vv = np.zeros(D)
    Ww = np.zeros(D)
    S2 = np.zeros(D)
    S1 = np.zeros(D)
    A = 0.0
    Bb = 0.0
    for r in results:
        Vv += r["o_v"].astype(np.float64)[0]
        Ww += r["o_w"].astype(np.float64)[0]
        S2 += r["o_s2"].astype(np.float64)[0]
        s1c = r["o_s1"].astype(np.float64)
        for b in range(BLKB):
            for dc in range(NDC):
                S1[128 * dc : 128 * (dc + 1)] += s1c[:, b * NDC + dc]
        A += float(r["o_ra"].astype(np.float64).sum())
        Bb += float(r["o_rb"].astype(np.float64).sum())
    m1 = S1 / N
    m2 = S2 / N
    S = A - 2.0 * Bb - float(np.dot(m2, Vv)) + 2.0 * float(np.dot(m1, Ww))
    return np.float32(-0.5 / N * S)


def run_on_device(x, p_mu, p_logvar, trace=False, **kw):
    from concourse import bass_utils

    nc = get_program()
    in_maps = make_in_maps(x, p_mu, p_logvar)
    return bass_utils.run_bass_kernel_spmd(
        nc, in_maps, list(range(NCORES)), trace=trace, **kw
    )


def kernel(x, p_mu, p_logvar):
    res = run_on_device(x, p_mu, p_logvar)
    return finish_host(res.results)


# revision 17
# speedup vs baseline: 1.1073x; 1.0678x over previous
"""CLUB loss kernel for Trainium2, 8-core data-parallel SPMD.

Math: with flat_x (N,D) [from x (B,D,H,W) -> (B*H*W, D)], v = exp(-p_logvar),
  loss = mean_i[ -0.5*sum_d ((x-mu)^2 - (m2 - 2*mu*m1 + mu^2)) * v ]
       = (-0.5/N) * [ A - 2B - dot(m2, V) + 2*dot(m1, W) ]
where
  A  = sum_{i,d} x^2 v          B  = sum_{i,d} x mu v
  V_d = sum_i v                 W_d = sum_i mu v
  m1 = S1/N, m2 = S2/N,  S1_d = sum_i x,  S2_d = sum_i x^2
All terms are per-core-local partial sums; the tiny (~KB) cross-core
reduction and final dot products happen on host in float64. No collectives.

Per-core device work (shard = 2048 rows = 2 b-blocks):
  - load x native (d-major), transpose 128x128 blocks on PE into PSUM
  - ACT: v = exp(-lv), p = square(xT), S1 via Copy+accum on native x
  - DVE: w = v*mu, fused scalar_tensor_tensor (mul+row-reduce) for A and B
  - PE: ones-matmul column sums for V, W, S2 in three PE column groups
    (separate PSUM banks, output partitions 0/32/64) accumulated over tiles
"""

import sys

import numpy as np

for _p in ("/opt/trn_rl_repo",):
    if _p not in sys.path:
        sys.path.append(_p)

B, D, H, W = 16, 512, 32, 32
HW = H * W
N = B * HW
NCORES = 8
BLKB = B // NCORES          # b-blocks per core (2)
ROWS = N // NCORES          # rows per core (2048)
NT = ROWS // 128            # 128-row tiles per core (16)
NDC = D // 128              # d chunks (4)
SLAB = 4                    # i-tiles per mu/lv DMA slab

_prog_cache = {}


def build_program():
    import concourse.bacc as bacc
    import concourse.tile as tile
    from concourse import masks, mybir

    f32 = mybir.dt.float32
    AF = mybir.ActivationFunctionType
    OP = mybir.AluOpType

    nc = bacc.Bacc(
        "TRN2",
        target_bir_lowering=False,
        debug=False,
        enable_asserts=False,
        num_devices=NCORES,
    )

    x_d = nc.dram_tensor("x_s", (BLKB, D, HW), f32, kind="ExternalInput").ap()
    mu_d = nc.dram_tensor("mu_s", (ROWS, D), f32, kind="ExternalInput").ap()
    lv_d = nc.dram_tensor("lv_s", (ROWS, D), f32, kind="ExternalInput").ap()

    # o_vws rows: 0 -> V, 1 -> W, 2 -> S2
    o_vws = nc.dram_tensor("o_vws", (3, D), f32, kind="ExternalOutput").ap()
    # o_misc cols: 0:NT -> rA, NT:2NT -> rB, 2NT:2NT+8 -> s1c
    o_misc = nc.dram_tensor(
        "o_misc", (128, 2 * NT + BLKB * NDC), f32, kind="ExternalOutput"
    ).ap()

    with tile.TileContext(nc) as tc:
        with (
            tc.tile_pool(name="const", bufs=1) as constp,
            tc.tile_pool(name="xnat", bufs=1) as xp,
            tc.tile_pool(name="slab", bufs=2) as slp,
            tc.tile_pool(name="stream", bufs=4) as sp,
            tc.tile_pool(name="accum", bufs=1) as accp,
            tc.tile_pool(name="scr", bufs=2) as scrp,
            tc.tile_pool(name="psum", bufs=2, space="PSUM") as pp,
            tc.tile_pool(name="psacc", bufs=1, space="PSUM") as pacc,
        ):
            ident = constp.tile([128, 128], f32)
            masks.make_identity(nc, ident[:])
            ones = constp.tile([128, 1], f32)
            nc.vector.memset(ones[:], 1.0)

            # three accumulator rows in three DIFFERENT psum banks, at
            # partitions 0/32/64 so their matmuls use distinct PE col groups
            v_acc = pacc.tile([1, D], f32, tag="v_acc")
            w_acc = pacc.tile([33, D], f32, tag="w_acc")
            s2_acc = pacc.tile([65, D], f32, tag="s2_acc")

            macc = accp.tile([128, 2 * NT + BLKB * NDC], f32, tag="macc")
            ra = macc[:, 0:NT]
            rb = macc[:, NT : 2 * NT]
            s1c = macc[:, 2 * NT : 2 * NT + BLKB * NDC]

            # ---- load x native: one 2 MiB DMA per b-block ----
            # x_sb[b][p, 1024*dc + hw] = x[b, 128*dc + p, hw]
            x_sb = []
            for b in range(BLKB):
                t_ = xp.tile([128, NDC * HW], f32, tag=f"x_{b}")
                nc.sync.dma_start(
                    t_[:], x_d[b].rearrange("(dc p) hw -> p dc hw", p=128)
                )
                x_sb.append(t_)

            # ---- S1: per-d sums of x over i, via ACT Copy + accum ----
            for b in range(BLKB):
                for dc in range(NDC):
                    scr_nat = scrp.tile([128, HW], f32, tag="scr_nat")
                    nc.scalar.activation(
                        scr_nat[:], x_sb[b][:, HW * dc : HW * (dc + 1)], AF.Copy,
                        accum_out=s1c[:, b * NDC + dc : b * NDC + dc + 1],
                    )

            # ---- main loop over 128-row i-tiles, mu/lv in 4-tile slabs ----
            mu_sl = lv_sl = None
            for t in range(NT):
                b, j = divmod(t, NT // BLKB)
                s, k = divmod(t, SLAB)
                if k == 0:
                    rows = mu_d[128 * SLAB * s : 128 * SLAB * (s + 1), :]
                    mu_sl = slp.tile([128, SLAB * D], f32, tag="mu_sl")
                    nc.sync.dma_start(
                        mu_sl[:], rows.rearrange("(g p) f -> p g f", p=128)
                    )
                    rows = lv_d[128 * SLAB * s : 128 * SLAB * (s + 1), :]
                    lv_sl = slp.tile([128, SLAB * D], f32, tag="lv_sl")
                    nc.sync.dma_start(
                        lv_sl[:], rows.rearrange("(g p) f -> p g f", p=128)
                    )
                mu_t = mu_sl[:, D * k : D * (k + 1)]
                lv_t = lv_sl[:, D * k : D * (k + 1)]

                xT = pp.tile([128, D], f32, tag="xT")
                for dc in range(NDC):
                    # 4 chunks share one PSUM bank: one accumulation group
                    nc.tensor.matmul(
                        xT[:, 128 * dc : 128 * (dc + 1)],
                        x_sb[b][:, HW * dc + 128 * j : HW * dc + 128 * (j + 1)],
                        ident[:],
                        is_transpose=True,
                        start=(dc == 0),
                        stop=(dc == NDC - 1),
                    )

                v_t = sp.tile([128, D], f32, tag="v")
                nc.scalar.activation(v_t[:], lv_t, AF.Exp, scale=-1.0)
                p_t = sp.tile([128, D], f32, tag="p")
                nc.scalar.activation(p_t[:], xT[:], AF.Square)

                w_t = sp.tile([128, D], f32, tag="w")
                nc.vector.tensor_tensor(w_t[:], v_t[:], mu_t, OP.mult)

                scr_a = scrp.tile([128, D], f32, tag="scr_a")
                nc.vector.scalar_tensor_tensor(
                    out=scr_a[:], in0=p_t[:], scalar=1.0, in1=v_t[:],
                    op0=OP.mult, op1=OP.mult,
                    accum_out=ra[:, t : t + 1],
                )
                scr_b = scrp.tile([128, D], f32, tag="scr_b")
                nc.vector.scalar_tensor_tensor(
                    out=scr_b[:], in0=w_t[:], scalar=1.0, in1=xT[:],
                    op0=OP.mult, op1=OP.mult,
                    accum_out=rb[:, t : t + 1],
                )

                st, sp_ = (t == 0), (t == NT - 1)
                nc.tensor.matmul(
                    v_acc[0:1, :], ones[:], v_t[:],
                    start=st, stop=sp_, tile_position=(0, 0),
                )
                nc.tensor.matmul(
                    w_acc[32:33, :], ones[:], w_t[:],
                    start=st, stop=sp_, tile_position=(0, 32),
                )
                nc.tensor.matmul(
                    s2_acc[64:65, :], ones[:], p_t[:],
                    start=st, stop=sp_, tile_position=(0, 64),
                )

            # ---- evacuate PSUM rows (lane-locked) -> SBUF -> DRAM ----
            rows_sb = accp.tile([65, D], f32, tag="rows_sb")
            nc.scalar.copy(rows_sb[0:1, :], v_acc[0:1, :])
            nc.scalar.copy(rows_sb[32:33, :], w_acc[32:33, :])
            nc.scalar.copy(rows_sb[64:65, :], s2_acc[64:65, :])

            nc.sync.dma_start(o_vws, rows_sb[0:65:32, :])
            nc.sync.dma_start(o_misc, macc[:])

    nc.compile()
    return nc


def get_program():
    if "nc" not in _prog_cache:
        _prog_cache["nc"] = build_program()
    return _prog_cache["nc"]


def make_in_maps(x, p_mu, p_logvar):
    x = np.ascontiguousarray(np.asarray(x, dtype=np.float32)).reshape(B, D, HW)
    p_mu = np.ascontiguousarray(np.asarray(p_mu, dtype=np.float32))
    p_logvar = np.ascontiguousarray(np.asarray(p_logvar, dtype=np.float32))
    in_maps = []
    for c in range(NCORES):
        in_maps.append(
            {
                "x_s": np.ascontiguousarray(x[BLKB * c : BLKB * (c + 1)]),
                "mu_s": np.ascontiguousarray(p_mu[ROWS * c : ROWS * (c + 1)]),
                "lv_s": np.ascontiguousarray(p_logvar[ROWS * c : ROWS * (c + 1)]),
            }
        )
    return in_maps


def finish_host(results):
    """Combine per-core partials (float64) into the scalar loss."""
    Vv = np.zeros(D)
    Ww = np.zeros(D)
    S2 = np.zeros(D)
    S1 = np.zeros(D)
    A = 0.0
    Bb = 0.0
    for r in results:
        vws = r["o_vws"].astype(np.float64)
        Vv += vws[0]
        Ww += vws[1]
        S2 += vws[2]
        misc = r["o_misc"].astype(np.float64)
        A += float(misc[:, 0:NT].sum())
        Bb += float(misc[:, NT : 2 * NT].sum())
        s1c = misc[:, 2 * NT :]
        for b in range(BLKB):
            for dc in range(NDC):
                S1[128 * dc : 128 * (dc + 1)] += s1c[:, b * NDC + dc]
    m1 = S1 / N
    m2 = S2 / N
    S = A - 2.0 * Bb - float(np.dot(m2, Vv)) + 2.0 * float(np.dot(m1, Ww))
    return np.float32(-0.5 / N * S)


def run_on_device(x, p_mu, p_logvar, trace=False, **kw):
    from concourse import bass_utils

    nc = get_program()
    in_maps = make_in_maps(x, p_mu, p_logvar)
    return bass_utils.run_bass_kernel_spmd(
        nc, in_maps, list(range(NCORES)), trace=trace, **kw
    )


def kernel(x, p_mu, p_logvar):
    res = run_on_device(x, p_mu, p_logvar)
    return finish_host(res.results)
